# revision 9
# baseline (speedup 1.0000x reference)
"""AL2Loss2d Trainium2 kernel (fp8 DoubleRow + fp16-2x one-hot).

Reference computation:
  inputs [8, 64, 512, 512] f32, targets [8, 512, 512] int64 (values 0..18)
  - per-class sums of the 64-dim pixel features (segment_sum over 2M pixels)
  - per-class counts
  - centers = sums / max(counts, 1); pairwise cosine similarity of the 19
    centers; CosineEmbeddingLoss-style reduction to a scalar.

Strategy: data-parallel over batch, one batch element per NeuronCore.
The rel-err budget (2e-2) is large, so the host ships features as
fp8_e4m3 (measured end-to-end rel err 5.7e-3), quartering HBM traffic
vs f32.

Per-core layout: pixels are packed [128 partitions, 1024 pairs, 2, 65]
fp8; the 65th column holds a per-class code mu[t] (19 distinct,
exactly-representable values), which doubles as the count feature:
accumulator column 64 = mu_k * count_k. Device pipeline per tile:
  - DMA tile (HBM streams ~430 B/ns when not backpressured)
  - Act: copies the code column to a parity-major fp16 buffer
  - DVE: fp16 one-hot [128, 2, 19, T] via is_equal with every operand
    2-byte packed stride-1 in the last dim -> DVE 2x_1port mode (the
    1-byte-out variant ran at 1x and throttled the whole pipeline)
  - PE: one DoubleRow fp8 matmul per pixel-pair (256 px / instr, 0.5
    cycles/row); the stationary one-hot is the fp8 view of the fp16
    tile at odd byte offsets (fp16 1.0 = 0x3C00 -> hot byte 0x3C =
    fp8 1.5, so every partial is uniformly scaled by 1.5; k-tile step
    19*2*T bytes is 16B-aligned as dual-fp8 Ldweights requires)
The tiny 19x19 cosine loss runs on host on the 8 gathered partials.
"""

import sys

import ml_dtypes
import numpy as np

if "/opt/trn_rl_repo" not in sys.path:
    sys.path.insert(0, "/opt/trn_rl_repo")

from concourse import bacc, bass, mybir, tile  # noqa: E402
from concourse.bass_utils import run_bass_kernel_spmd  # noqa: E402

K = 19
CH = 64
CW = CH + 1  # 64 channel sums | mu-scaled count column
NCORES = 8
NPART = 128
EPS = 1e-8
NPAIR = 1024  # 2048 px per partition = 1024 DoubleRow pairs
PADJ = 1  # pad pair: keeps the HBM partition stride off large pow2 multiples
OHSCALE = 1.5  # fp8 value of the hot byte of fp16 1.0

FP8 = ml_dtypes.float8_e4m3
# 19 distinct per-class codes, all exactly representable in e4m3 (and
# fp16) so the count column mu_k * count_k divides back exactly.
MU = np.array(
    [1, 2, 3, 4, 5, 6, 7, 8, 9, 10, 11, 12, 13, 14, 15, 16, 18, 20, 22],
    dtype=np.float32,
)
MU_FP8 = MU.astype(FP8)
assert np.all(MU_FP8.astype(np.float32) == MU)


def pair_segments(npair: int, g: int):
    """Ramp-up -> main tiles of g pairs -> tapered tail.

    Small leading tiles start the Act/DVE/PE pipeline as soon as the
    first bytes land (a leading full-size tile delays the first matmul
    by its whole DMA+onehot latency); small trailing tiles shrink the
    compute left after the last DMA byte.
    """
    ramp = [max(1, g // 16), max(1, g // 16), g // 8, g // 4, g // 2]
    tail = [g // 2, g // 4, g // 8, max(1, g // 16), max(1, g // 16)]
    ramp = [t for t in ramp if t > 0]
    tail = [t for t in tail if t > 0]
    if npair <= sum(ramp) + sum(tail):
        # tiny builds (sim tests): single pass of g-sized tiles
        segs = []
        j = 0
        while j < npair:
            t = min(g, npair - j)
            segs.append((j, t))
            j += t
        return segs
    segs = []
    j = 0
    for t in ramp:
        segs.append((j, t))
        j += t
    while npair - j > sum(tail):
        t = min(g, npair - j - sum(tail))
        segs.append((j, t))
        j += t
    for t in tail:
        segs.append((j, t))
        j += t
    assert sum(s[1] for s in segs) == npair, segs
    return segs


def build(npair: int, g: int) -> bass.Bass:
    """Per-core Bass program (pixels = 128 * npair * 2)."""
    segs = pair_segments(npair, g)
    nc = bacc.Bacc(target_bir_lowering=False, trn_type="TRN2")
    x_ext = nc.declare_dram_parameter(
        "x", [NPART, npair + PADJ, 2, CW], mybir.dt.float8e4, isOutput=False
    )
    mu_ext = nc.declare_dram_parameter(
        "murep", [NPART, K, g], mybir.dt.float16, isOutput=False
    )
    out_ext = nc.declare_dram_parameter("out", [K, CW], mybir.dt.float32, isOutput=True)

    with tile.TileContext(nc) as tc:
        with (
            tc.tile_pool(name="const", bufs=1) as cpool,
            tc.tile_pool(name="xin", bufs=5) as xpool,
            tc.tile_pool(name="tq", bufs=5) as tqpool,
            tc.tile_pool(name="oh", bufs=5) as ohpool,
            tc.tile_pool(name="acc", bufs=1, space=bass.MemorySpace.PSUM) as psumpool,
            tc.tile_pool(name="outp", bufs=1) as opool,
        ):
            # per-class code table replicated along the pair axis (tiny DMA
            # on the Act queue so it never waits behind an x tile)
            mu_sb = cpool.tile([NPART, K, g], mybir.dt.float16)
            nc.scalar.dma_start(mu_sb[:], mu_ext[:])

            acc = psumpool.tile([K, CW], mybir.dt.float32)
            nmm = npair
            mm = 0
            for j0, gg in segs:
                xt = xpool.tile([NPART, g, 2, CW], mybir.dt.float8e4, tag="xt")
                nc.sync.dma_start(xt[:, :gg], x_ext[:, j0 : j0 + gg])
                # Act: parity-major fp16 copy of the class-code column
                t16 = tqpool.tile([NPART, 2, g], mybir.dt.float16, tag="t16")
                nc.scalar.copy(
                    t16[:, :, :gg], xt[:, :gg, :, CH].transpose([0, 2, 1])
                )
                # DVE: fp16 one-hot, all operands 2-byte packed -> 2x_1p
                oh = ohpool.tile([NPART, 2, K, g], mybir.dt.float16, tag="oh")
                in0 = t16[:, :, :gg].unsqueeze(2).broadcast_to([NPART, 2, K, gg])
                in1 = mu_sb[:, :, :gg].unsqueeze(1).broadcast_to([NPART, 2, K, gg])
                nc.vector.tensor_tensor(
                    out=oh[:, :, :, :gg],
                    in0=in0,
                    in1=in1,
                    op=mybir.AluOpType.is_equal,
                )
                # PE: hot byte of fp16 1.0 at odd offsets -> fp8 1.5 one-hot
                oh8 = oh[:].bitcast(mybir.dt.float8e4)  # [128, 2, K, 2g]
                for j in range(gg):
                    nc.tensor.matmul(
                        acc[:],
                        oh8[:, :, :, 2 * j + 1],
                        xt[:, j],
                        start=(mm == 0),
                        stop=(mm == nmm - 1),
                        perf_mode=mybir.MatmulPerfMode.DoubleRow,
                    )
                    mm += 1
            out_sb = opool.tile([K, CW], mybir.dt.float32)
            nc.vector.tensor_copy(out_sb[:], acc[:])
            nc.sync.dma_start(out_ext[:], out_sb[:])
    nc.compile()
    return nc


def prep_shard(xq_b: np.ndarray, t_b: np.ndarray, npair: int):
    """xq_b [64, H, W] fp8, t_b [H, W] int -> device x array."""
    npix = t_b.size
    xr = xq_b.reshape(CH, NPART, npix // NPART).transpose(1, 2, 0)
    xdev = np.zeros((NPART, npair + PADJ, 2, CW), dtype=FP8)
    xdev[:, :npair, :, :CH] = xr.reshape(NPART, npair, 2, CH)
    tcode = MU_FP8[t_b.reshape(NPART, npix // NPART)]
    xdev[:, :npair, :, CH] = tcode.reshape(NPART, npair, 2)
    return xdev


def make_murep(g: int) -> np.ndarray:
    return np.broadcast_to(
        MU.astype(np.float16)[None, :, None], (NPART, K, g)
    ).copy()


_NC_CACHE: dict = {}
TRACE = False  # set True (e.g. from test.py) to profile; result lands here
LAST_RESULT = None
G = 128  # pairs per tile


def _get_nc(npair: int) -> bass.Bass:
    key = (npair, G)
    if key not in _NC_CACHE:
        _NC_CACHE[key] = build(npair, G)
    return _NC_CACHE[key]


def finish(partials: np.ndarray) -> np.float32:
    """partials [ncores, K, CW] -> scalar loss (host, mirrors reference)."""
    total = partials.sum(axis=0, dtype=np.float64) / OHSCALE
    sums = total[:, :CH]
    counts = total[:, CH] / MU.astype(np.float64)
    centers = sums / np.maximum(counts, 1.0)[:, None]
    norms = np.maximum(np.sqrt((centers * centers).sum(axis=1)), EPS)
    cn = centers / norms[:, None]
    S = cn @ cn.T
    eye = np.eye(K, dtype=bool)
    per_pair = np.where(eye, 1.0 - S, np.maximum(S, 0.0))
    return np.float32(per_pair.sum() / (K * K * K))


def kernel(inputs: np.ndarray, targets: np.ndarray) -> np.ndarray:
    B, C, H, W = inputs.shape
    assert (B, C) == (NCORES, CH)
    npair = H * W // NPART // 2
    nc = _get_nc(npair)

    xq = np.asarray(inputs).astype(FP8)
    tgt = np.asarray(targets)
    murep = make_murep(G)
    in_maps = []
    for i in range(NCORES):
        xdev = prep_shard(xq[i], tgt[i], npair)
        in_maps.append({"x": xdev, "murep": murep})

    res = run_bass_kernel_spmd(
        nc, in_maps, core_ids=list(range(NCORES)), trace=TRACE
    )
    global LAST_RESULT
    LAST_RESULT = res
    partials = np.stack([r["out"] for r in res.results])
    return np.asarray(finish(partials))
